# revision 2
# baseline (speedup 1.0000x reference)
"""Trainium2 Bass kernel for nn_LiquidNeuralNetwork.

Strategy: data-parallel over batch (8 cores x 64 batch). Per core, a fully
on-chip recurrence over T=512 steps. Layers are software-pipelined one time
step apart (layer0 at t, layer1 at t-1) and packed side-by-side in the free
dim so elementwise/activation ops cover both layers in one instruction.

Layout: compute tiles are [128 partitions = h%128, free = l*128 + q*64 + b]
(l = layer stream, q = H half, b = batch-in-core). All linear algebra of the
LTC cell (input projections, recurrent matmuls, -h/tau diagonal terms, RK4
hh-increments) is accumulated in PSUM by the tensor engine using bf16
weights; only the gate nonlinearity (tanh, sigmoid on ScalarE), the gated
product and the RK4 k-assembly run on the vector engines. LayerNorm is done
with ones-matmul partition reductions, a Newton-iteration rsqrt on the DVE
(no sqrt activation table thrash), and rank-1/rank-2 matmul broadcasts that
fold gamma/beta/mu/rstd into a single multiply-add before the output tanh.
"""

import os
import sys
import numpy as np

sys.path.insert(0, "/opt/trn_rl_repo")

B, T, FS, FC, H, L = 512, 512, 64, 32, 256, 2
LN_EPS = 1e-5
NCORES = 8
BC = B // NCORES       # 64 batch per core
NF = 2 * H * BC // 128  # 256 free size of packed tiles
UNROLL = 8
# Newton rsqrt seed: y0 = 1/(a + b*vv), vv = N^2*(var+36eps), guaranteed
# underestimate when 4ab >= 1. Calibrated for vv in [VLO, VHI].
N_H = 256.0
RSQRT_ITERS = 3


def _seed_consts(vlo, vhi):
    vc = float(np.sqrt(vlo * vhi))
    b = 0.5 / np.sqrt(vc)
    a = 1.0 / (4.0 * b)
    return a, b


# vv = N^2 * (var(P) + 36eps); var(P) observed ~[1, 400] (P = 6*hn_pre), with
# big safety margin -> vv range [256^2*0.05, 256^2*4000]
SEED_A, SEED_B = _seed_consts(256.0 * 256.0 * 0.05, 256.0 * 256.0 * 4000.0)


def softplus_np(x):
    return np.log1p(np.exp(-np.abs(x))) + np.maximum(x, 0)


def _bf16(x):
    import ml_dtypes
    return np.asarray(x, np.float32).astype(ml_dtypes.bfloat16)


# ---------------------------------------------------------------------------
# Bass module builder
# ---------------------------------------------------------------------------

def build_module(T_run=T, unroll=UNROLL):
    import concourse.bass as bass
    import concourse.mybir as mybir
    from concourse import tile, bacc
    from concourse.bass import ds

    f32 = mybir.dt.float32
    bf16 = mybir.dt.bfloat16
    AF = mybir.ActivationFunctionType
    OP = mybir.AluOpType

    nc = bacc.Bacc(None, target_bir_lowering=False)

    # ---- DRAM I/O -------------------------------------------------------
    xcat = nc.declare_dram_parameter("xcat", [T_run, 97, BC], bf16, isOutput=False)
    # weights, pre-transposed/packed on host (see _prep_weights)
    wdecl = {}

    def wparam(name, shape, dtype=bf16):
        wdecl[name] = nc.declare_dram_parameter(name, shape, dtype, isOutput=False)
        return wdecl[name]

    # layer0: K-dims: x-part 97 (seq 64 + ctx 32 + bias 1), h-part 256
    wparam("g_x0", [97, 256])           # [Wg0_x.T ; bg0] , M=(q*128)
    wparam("in_x0", [96, 256])          # Win0_cat.T
    wparam("g_h0", [128, 512])          # (kt, m*128): Wg0_h.T tiles
    wparam("g_h0h", [128, 512])         # x0.5
    wparam("rec0", [128, 512])
    wparam("rec0h", [128, 512])
    wparam("dneg0", [128, 256])         # per q: diag(-itau0)
    wparam("dneg0h", [128, 256])        # per q: diag(-0.5*itau0)
    # layer1: x-part = h0 (K=256), h-part 256
    wparam("g_x1", [128, 512])
    wparam("bg1row", [1, 256])          # bias row, M=(q*128)
    wparam("in_x1", [128, 512])
    wparam("g_h1", [128, 512])
    wparam("g_h1h", [128, 512])
    wparam("rec1", [128, 512])
    wparam("rec1h", [128, 512])
    wparam("dneg1", [128, 256])
    wparam("dneg1h", [128, 256])
    # P-build identity variants
    wparam("id6", [128, 128])
    wparam("id1", [128, 128])
    wparam("id2", [128, 128])
    # LN: per (l,q): row0=lnb, row1=-lng ; and lngN rows (lng*N) for s'
    wparam("lnT", [2, 512])             # [(l*2+q)*128]
    wparam("lngN", [1, 512])
    wparam("ones_red", [128, 1])
    wparam("ones2", [2, 128])
    hout = nc.declare_dram_parameter("hout", [128, 128], bf16, isOutput=True)

    NITER = (T_run - 1 - ((T_run - 1) % unroll)) // unroll  # loop covers ticks 1..NITER*unroll
    loop_hi = 1 + NITER * unroll

    with tile.TileContext(nc) as tc:
        from contextlib import ExitStack
        with ExitStack() as ctx:
            singles = ctx.enter_context(tc.tile_pool(name="singles", bufs=1))
            xc_pool = ctx.enter_context(tc.tile_pool(name="xc", bufs=3))
            g_pool = ctx.enter_context(tc.tile_pool(name="g", bufs=2))
            k_pool = ctx.enter_context(tc.tile_pool(name="k", bufs=3))
            s_pool = ctx.enter_context(tc.tile_pool(name="s", bufs=3))
            row_pool = ctx.enter_context(tc.tile_pool(name="row", bufs=4))
            g_psum = ctx.enter_context(tc.tile_pool(name="gps", bufs=1, space="PSUM"))
            r_psum = ctx.enter_context(tc.tile_pool(name="rps", bufs=1, space="PSUM"))
            v_psum = ctx.enter_context(tc.tile_pool(name="vps", bufs=1, space="PSUM"))
            p_psum = ctx.enter_context(tc.tile_pool(name="pps", bufs=1, space="PSUM"))
            st_psum = ctx.enter_context(tc.tile_pool(name="stp", bufs=2, space="PSUM"))
            bc_psum = ctx.enter_context(tc.tile_pool(name="bcp", bufs=2, space="PSUM"))

            # ---- load weights to SBUF (resident) -------------------------
            W = {}
            for name, dram in wdecl.items():
                t_ = singles.tile(list(dram.shape), dram.dtype, name=name, tag=name)
                nc.sync.dma_start(t_[:], dram[:])
                W[name] = t_

            # ring of state tiles; slot(tau) = tau % 8 holds output of tick tau
            ring = [singles.tile([128, 256], bf16, name=f"ring{i}", tag=f"ring{i}") for i in range(8)]
            hz = singles.tile([128, 256], bf16)   # zero state
            nc.vector.memset(hz[:], 0.0)
            trhs = singles.tile([2, 128], bf16)   # rank-2 rhs; row0=nm, row1=1
            nc.sync.dma_start(trhs[:, :], wdecl["ones2"][:, :])
            onesr = singles.tile([1, 64], bf16)   # ones rhs row for bg1
            nc.vector.memset(onesr[:], 1.0)

            def slot(tau):
                return ring[tau % 8]

            # W slicing helpers: packed (kt, m) tiles
            def wt(name, kt, m):
                return W[name][:, kt * 256 + m * 128: kt * 256 + (m + 1) * 128]

            def dneg(l, half, m):
                nm = f"dneg{l}" + ("h" if half else "")
                return W[nm][:, m * 128:(m + 1) * 128]

            # free-dim column helpers for packed [128, 256] tiles
            def col(l, q):
                return slice(l * 128 + q * 64, l * 128 + (q + 1) * 64)

            def emit_tick(xc, h_prev, h1_prev, h_next, do0, do1):
                """h_prev: state tile holding (h0[t-1] | h1[t-2]-ish l0 half
                used for both L0 state and L1 input; h1_prev: tile whose l1
                half is layer1's state. h_next: output tile."""
                lo = 0 if do0 else 128
                hi = 256 if do1 else 128
                cs = slice(lo, hi)          # full packed range this tick
                l_first = 0 if do0 else 1
                l_last = 1 if do1 else 0
                # -------- per-step accumulation banks (one group each) ---
                G = g_psum.tile([128, 256], f32, tag="G")
                R = r_psum.tile([128, 256], f32, tag="R")
                V = v_psum.tile([128, 256], f32, tag="V")
                _first = {}

                def bmm(bank, out_ap, lhsT, rhs, last=False):
                    st_ = bank not in _first
                    _first[bank] = True
                    nc.tensor.matmul(out_ap, lhsT, rhs, start=st_, stop=last, skip_group_check=True)
                # G/R/V initial (stage-1) content
                if do0:
                    for m in range(2):
                        bmm("G", G[:, col(0, m)], W["g_x0"][:, m * 128:(m + 1) * 128], xc[:, :])
                        bmm("G", G[:, col(0, m)], wt("g_h0", 0, m), h_prev[:, 0:64])
                        bmm("G", G[:, col(0, m)], wt("g_h0", 1, m), h_prev[:, 64:128])
                        bmm("R", R[:, col(0, m)], wt("rec0", 0, m), h_prev[:, 0:64])
                        bmm("R", R[:, col(0, m)], wt("rec0", 1, m), h_prev[:, 64:128])
                        bmm("V", V[:, col(0, m)], W["in_x0"][:, m * 128:(m + 1) * 128], xc[0:96, :])
                        bmm("V", V[:, col(0, m)], dneg(0, False, m), h_prev[:, m * 64:(m + 1) * 64])
                if do1:
                    x1 = h_prev  # l0 half = h0[t-1] = layer1 input
                    for m in range(2):
                        bmm("G", G[:, col(1, m)], wt("g_x1", 0, m), x1[:, 0:64])
                        bmm("G", G[:, col(1, m)], wt("g_x1", 1, m), x1[:, 64:128])
                        bmm("G", G[:, col(1, m)], W["bg1row"][:, m * 128:(m + 1) * 128], onesr[:, :])
                        bmm("G", G[:, col(1, m)], wt("g_h1", 0, m), h1_prev[:, 128:192])
                        bmm("G", G[:, col(1, m)], wt("g_h1", 1, m), h1_prev[:, 192:256])
                        bmm("R", R[:, col(1, m)], wt("rec1", 0, m), h1_prev[:, 128:192])
                        bmm("R", R[:, col(1, m)], wt("rec1", 1, m), h1_prev[:, 192:256])
                        bmm("V", V[:, col(1, m)], wt("in_x1", 0, m), x1[:, 0:64])
                        bmm("V", V[:, col(1, m)], wt("in_x1", 1, m), x1[:, 64:128])
                        bmm("V", V[:, col(1, m)], dneg(1, False, m), h1_prev[:, 128 + m * 64:128 + (m + 1) * 64])

                ks = []
                P_sb = s_pool.tile([128, 256], bf16, tag="P")
                for s in range(4):
                    if s > 0:
                        # Delta rhs and weight-variant selection
                        if s == 1:
                            dr = ks[0]
                            halfw = True
                        elif s == 2:
                            dr = k_pool.tile([128, 256], bf16, tag="d")
                            nc.vector.scalar_tensor_tensor(out=dr[:, cs], in0=ks[0][:, cs], scalar=-1.0, in1=ks[1][:, cs], op0=OP.mult, op1=OP.add)
                            halfw = True
                        else:
                            dr = k_pool.tile([128, 256], bf16, tag="d")
                            # k3 - 0.5*k2 via cody_waite cascade (1 op)
                            nc.vector.scalar_tensor_tensor(out=dr[:, cs], in0=ks[1][:, cs], scalar=-0.5, in1=ks[2][:, cs], op0=OP.mult, op1=OP.add)
                            halfw = False
                        sfx = "h" if halfw else ""
                        for l in range(l_first, l_last + 1):
                            for m in range(2):
                                last = (s == 3) and (l == l_last) and (m == 1)
                                bmm("G", G[:, col(l, m)], wt(f"g_h{l}{sfx}", 0, m), dr[:, l * 128:l * 128 + 64])
                                bmm("G", G[:, col(l, m)], wt(f"g_h{l}{sfx}", 1, m), dr[:, l * 128 + 64:l * 128 + 128], last=last)
                                bmm("R", R[:, col(l, m)], wt(f"rec{l}{sfx}", 0, m), dr[:, l * 128:l * 128 + 64])
                                bmm("R", R[:, col(l, m)], wt(f"rec{l}{sfx}", 1, m), dr[:, l * 128 + 64:l * 128 + 128], last=last)
                                bmm("V", V[:, col(l, m)], dneg(l, halfw, m), dr[:, l * 128 + m * 64:l * 128 + (m + 1) * 64], last=last)
                    # gate and k
                    tg = g_pool.tile([128, 256], f32, tag="tg")
                    nc.scalar.activation(tg[:, cs], G[:, cs], AF.Tanh)
                    gg = g_pool.tile([128, 256], f32, tag="gg")
                    nc.scalar.activation(gg[:, cs], tg[:, cs], AF.Sigmoid)
                    mm = g_pool.tile([128, 256], f32, tag="mm")
                    nc.vector.scalar_tensor_tensor(out=mm[:, cs], in0=gg[:, cs], scalar=0.0, in1=R[:, cs], op0=OP.add, op1=OP.mult)
                    kk = k_pool.tile([128, 256], bf16, tag=f"k{s}")
                    nc.vector.scalar_tensor_tensor(out=kk[:, cs], in0=mm[:, cs], scalar=0.0, in1=V[:, cs], op0=OP.add, op1=OP.add)
                    ks.append(kk)

                # -------- P = 6h + k1 + 2k2 + 2k3 + k4 (PSUM, PE) --------
                PP = p_psum.tile([128, 256], f32, tag="PP")
                for l in range(l_first, l_last + 1):
                    hsrc = h_prev if l == 0 else h1_prev
                    for q in range(2):
                        c = col(l, q)
                        rs = slice(l * 128 + q * 64, l * 128 + (q + 1) * 64)
                        plast = (l == l_last) and (q == 1)
                        bmm("P", PP[:, c], W["id6"][:, :], hsrc[:, rs])
                        bmm("P", PP[:, c], W["id1"][:, :], ks[0][:, rs])
                        bmm("P", PP[:, c], W["id2"][:, :], ks[1][:, rs])
                        bmm("P", PP[:, c], W["id2"][:, :], ks[2][:, rs])
                        bmm("P", PP[:, c], W["id1"][:, :], ks[3][:, rs], last=plast)
                nc.scalar.activation(P_sb[:, cs], PP[:, cs], AF.Copy)
                P2_sb = s_pool.tile([128, 256], bf16, tag="P2")
                nc.scalar.activation(P2_sb[:, cs], PP[:, cs], AF.Square)

                # -------- LN stats: sums over H via ones-matmul ----------
                stp = st_psum.tile([1, 512], f32, tag="st")
                for l in range(l_first, l_last + 1):
                    slast = (l == l_last)
                    bmm("S", stp[:, l * 64:(l + 1) * 64], W["ones_red"][:, :], P_sb[:, col(l, 0)])
                    bmm("S", stp[:, l * 64:(l + 1) * 64], W["ones_red"][:, :], P_sb[:, col(l, 1)])
                    bmm("S", stp[:, 256 + l * 64:256 + (l + 1) * 64], W["ones_red"][:, :], P2_sb[:, col(l, 0)])
                    bmm("S", stp[:, 256 + l * 64:256 + (l + 1) * 64], W["ones_red"][:, :], P2_sb[:, col(l, 1)], last=slast)
                rlo = 0 if do0 else 64
                rhi = 128 if do1 else 64
                rs_ = slice(rlo, rhi)
                sx = row_pool.tile([1, 128], f32, tag="sx")
                nc.vector.tensor_scalar(out=sx[:, rs_], in0=stp[0:1, rs_], scalar1=1.0, scalar2=None, op0=OP.mult)
                ms = row_pool.tile([1, 128], f32, tag="ms")
                nc.vector.scalar_tensor_tensor(out=ms[:, rs_], in0=sx[:, rs_], scalar=0.0, in1=sx[:, rs_], op0=OP.add, op1=OP.mult)
                t1 = row_pool.tile([1, 128], f32, tag="t1")
                nc.vector.tensor_scalar(out=t1[:, rs_], in0=stp[0:1, 256 + rlo:256 + rhi], scalar1=N_H, scalar2=N_H * N_H * 36.0 * LN_EPS, op0=OP.mult, op1=OP.add)
                vv = row_pool.tile([1, 128], f32, tag="vv")
                nc.vector.scalar_tensor_tensor(out=vv[:, rs_], in0=ms[:, rs_], scalar=-1.0, in1=t1[:, rs_], op0=OP.mult, op1=OP.add)
                w_ = row_pool.tile([1, 128], f32, tag="w")
                nc.vector.tensor_scalar(out=w_[:, rs_], in0=vv[:, rs_], scalar1=SEED_B, scalar2=SEED_A, op0=OP.mult, op1=OP.add)
                y = row_pool.tile([1, 128], f32, tag="y")
                yb = row_pool.tile([1, 128], bf16, tag="yb")
                nc.vector.reciprocal_approx_fast(out=y[:, rs_], in_=w_[:, rs_])
                yy = row_pool.tile([1, 128], f32, tag="yy")
                tn = row_pool.tile([1, 128], f32, tag="tn")
                for _ in range(RSQRT_ITERS):
                    nc.vector.scalar_tensor_tensor(out=yy[:, rs_], in0=y[:, rs_], scalar=0.0, in1=y[:, rs_], op0=OP.add, op1=OP.mult)
                    nc.vector.scalar_tensor_tensor(out=tn[:, rs_], in0=yy[:, rs_], scalar=-0.5, in1=vv[:, rs_], op0=OP.mult, op1=OP.mult)
                    nc.vector.scalar_tensor_tensor(out=y[:, rs_], in0=tn[:, rs_], scalar=1.5, in1=y[:, rs_], op0=OP.add, op1=OP.mult)
                nc.vector.tensor_scalar(out=yb[:, rs_], in0=y[:, rs_], scalar1=0.0, scalar2=None, op0=OP.add)
                # nm = sum(x)*y -> row1 of trhs (t' = lnb + (-lng)*mu*rstd)
                nc.vector.scalar_tensor_tensor(out=trhs[0:1, rs_], in0=sx[:, rs_], scalar=0.0, in1=y[:, rs_], op0=OP.add, op1=OP.mult)

                # -------- broadcasts: s' = lngN (x) y ; t' = lnb(x)1 + (-lng)(x)nm
                bcp = bc_psum.tile([128, 512], f32, tag="bc")
                for l in range(l_first, l_last + 1):
                    for q in range(2):
                        lq = (2 * l + q) * 128
                        blast = (l == l_last) and (q == 1)
                        bmm("B", bcp[:, col(l, q)], W["lngN"][:, lq:lq + 128], yb[0:1, l * 64:(l + 1) * 64])
                        bmm("B", bcp[:, 256 + l * 128 + q * 64:256 + l * 128 + (q + 1) * 64], W["lnT"][:, lq:lq + 128], trhs[:, l * 64:(l + 1) * 64], last=blast)
                z = s_pool.tile([128, 256], f32, tag="z")
                nc.vector.scalar_tensor_tensor(out=z[:, cs], in0=P_sb[:, cs], scalar=0.0, in1=bcp[:, lo:hi], op0=OP.add, op1=OP.mult)
                z2 = s_pool.tile([128, 256], f32, tag="z2")
                nc.vector.scalar_tensor_tensor(out=z2[:, cs], in0=z[:, cs], scalar=0.0, in1=bcp[:, 256 + lo:256 + hi], op0=OP.add, op1=OP.add)
                nc.scalar.activation(h_next[:, cs], z2[:, cs], AF.Tanh)

            # ----- tick 0: layer0 only, zero states; zero l1 half of slot0
            xc0 = xc_pool.tile([97, BC], bf16, tag="xc")
            nc.sync.dma_start(xc0[:], xcat[0])
            nc.vector.memset(ring[0][:, 128:256], 0.0)
            emit_tick(xc0, hz, hz, ring[0], True, False)

            # ----- main loop: ticks 1 .. loop_hi-1 ------------------------
            with tc.For_i(1, loop_hi, unroll) as iv:
                xslab = xcat[ds(iv, unroll)]
                for u in range(unroll):
                    tau = u + 1  # slot parity: tick tau=1+8j+u -> reads slot((u)%8) writes slot((u+1)%8)
                    xcu = xc_pool.tile([97, BC], bf16, tag="xc")
                    nc.sync.dma_start(xcu[:], xslab[u])
                    emit_tick(xcu, ring[u % 8], ring[u % 8], ring[(u + 1) % 8], True, True)

            # ----- tail ticks: loop_hi .. T_run ---------------------------
            for tau in range(loop_hi, T_run + 1):
                do0 = tau < T_run
                if do0:
                    xct = xc_pool.tile([97, BC], bf16, tag="xc")
                    nc.sync.dma_start(xct[:], xcat[tau])
                else:
                    xct = None
                emit_tick(xct, slot(tau - 1), slot(tau - 1), slot(tau), do0, True)

            nc.sync.dma_start(hout[:], slot(T_run)[:, 128:256])

    nc.compile()
    return nc


# ---------------------------------------------------------------------------
# Host-side weight prep
# ---------------------------------------------------------------------------

def _prep_weights(inputs):
    import ml_dtypes
    bf = ml_dtypes.bfloat16
    W = {k: np.asarray(v, np.float32) for k, v in inputs.items()}
    out = {}

    def pack_kt(wT):  # [256, 256] -> [128, 512] (kt, m)
        return np.concatenate([wT[0:128, :], wT[128:256, :]], axis=1)

    for l in range(L):
        fin = FS + FC if l == 0 else H
        Wg, Win, Wrec = W[f'Wg{l}'], W[f'Win{l}'], W[f'Wrec{l}']
        bg, tau = W[f'bg{l}'], W[f'tau{l}']
        itau = (1.0 / (softplus_np(tau) + 1.0)).astype(np.float32)
        WgxT = Wg[:, :fin].T            # [fin, 256]
        WghT = Wg[:, fin:].T            # [256, 256]
        WrecT = Wrec.T
        WinT = Win.T                    # [fin, 256]
        if l == 0:
            out["g_x0"] = np.concatenate([WgxT, bg[None, :]], 0).astype(bf)   # [97, 256]
            out["in_x0"] = WinT.astype(bf)                                     # [96, 256]
            out["g_h0"] = pack_kt(WghT).astype(bf)
            out["g_h0h"] = pack_kt(WghT * 0.5).astype(bf)
            out["rec0"] = pack_kt(WrecT).astype(bf)
            out["rec0h"] = pack_kt(WrecT * 0.5).astype(bf)
            dn = np.zeros((128, 256), np.float32)
            dnh = np.zeros((128, 256), np.float32)
            for q in range(2):
                dn[:, q * 128:(q + 1) * 128] = np.diag(-itau[q * 128:(q + 1) * 128])
                dnh[:, q * 128:(q + 1) * 128] = np.diag(-0.5 * itau[q * 128:(q + 1) * 128])
            out["dneg0"] = dn.astype(bf)
            out["dneg0h"] = dnh.astype(bf)
        else:
            out["g_x1"] = pack_kt(WgxT).astype(bf)
            out["bg1row"] = bg[None, :].astype(bf)
            out["in_x1"] = pack_kt(WinT).astype(bf)
            out["g_h1"] = pack_kt(WghT).astype(bf)
            out["g_h1h"] = pack_kt(WghT * 0.5).astype(bf)
            out["rec1"] = pack_kt(WrecT).astype(bf)
            out["rec1h"] = pack_kt(WrecT * 0.5).astype(bf)
            dn = np.zeros((128, 256), np.float32)
            dnh = np.zeros((128, 256), np.float32)
            for q in range(2):
                dn[:, q * 128:(q + 1) * 128] = np.diag(-itau[q * 128:(q + 1) * 128])
                dnh[:, q * 128:(q + 1) * 128] = np.diag(-0.5 * itau[q * 128:(q + 1) * 128])
            out["dneg1"] = dn.astype(bf)
            out["dneg1h"] = dnh.astype(bf)
    out["id6"] = (6.0 * np.eye(128, dtype=np.float32)).astype(bf)
    out["id1"] = np.eye(128, dtype=np.float32).astype(bf)
    out["id2"] = (2.0 * np.eye(128, dtype=np.float32)).astype(bf)
    lnT = np.zeros((2, 512), np.float32)
    lngN = np.zeros((1, 512), np.float32)
    for l in range(L):
        lng, lnb = W[f'lng{l}'], W[f'lnb{l}']
        for q in range(2):
            lq = (2 * l + q) * 128
            lnT[0, lq:lq + 128] = -lng[q * 128:(q + 1) * 128]
            lnT[1, lq:lq + 128] = lnb[q * 128:(q + 1) * 128]
            lngN[0, lq:lq + 128] = lng[q * 128:(q + 1) * 128] * N_H
    out["lnT"] = lnT.astype(bf)
    out["lngN"] = lngN.astype(bf)
    out["ones_red"] = np.ones((128, 1), np.float32).astype(bf)
    out["ones2"] = np.ones((2, 128), np.float32).astype(bf)
    return out


def _prep_core_inputs(inputs, wpack, core, T_run=T):
    seq = np.asarray(inputs['seq_features'], np.float32)   # [B,T,FS]
    ctx = np.asarray(inputs['context_features'], np.float32)  # [B,FC]
    bsl = slice(core * BC, (core + 1) * BC)
    import ml_dtypes
    xc = np.empty((T_run, 97, BC), np.float32)
    xc[:, 0:64, :] = seq[bsl, :T_run].transpose(1, 2, 0)     # [T, FS, BC]
    xc[:, 64:96, :] = ctx[bsl].T[None, :, :]
    xc[:, 96, :] = 1.0
    m = {"xcat": xc.astype(ml_dtypes.bfloat16)}
    m.update(wpack)
    return m


def _head(inputs, h1):  # h1: [B, H] final layer1 state
    cW1 = np.asarray(inputs['cW1'], np.float32)
    cb1 = np.asarray(inputs['cb1'], np.float32)
    cW2 = np.asarray(inputs['cW2'], np.float32)
    cb2 = np.asarray(inputs['cb2'], np.float32)
    hid = np.maximum(h1 @ cW1.T + cb1, 0)
    return (hid @ cW2.T + cb2).squeeze(-1)


_CACHE = {}


def kernel(**inputs):
    if "nc" not in _CACHE:
        _CACHE["nc"] = build_module(T, UNROLL)
    nc = _CACHE["nc"]
    from concourse.bass_utils import run_bass_kernel_spmd
    wpack = _prep_weights(inputs)
    in_maps = [_prep_core_inputs(inputs, wpack, c) for c in range(NCORES)]
    out = run_bass_kernel_spmd(nc, in_maps, list(range(NCORES)))
    res = out.results
    if out.exec_time_ns:
        _CACHE["exec_ns"] = out.exec_time_ns
    if out.instructions_and_trace:
        _CACHE["trace_path"] = out.instructions_and_trace[1]
    h1 = np.empty((B, H), np.float32)
    for c in range(NCORES):
        ht = np.asarray(res[c]["hout"], np.float32)  # [128, (q,b)]
        bsl = slice(c * BC, (c + 1) * BC)
        for q in range(2):
            h1[bsl, q * 128:(q + 1) * 128] = ht[:, q * 64:(q + 1) * 64].T
    return _head(inputs, h1).astype(np.float32)


if __name__ == "__main__":
    pass

